# revision 10
# baseline (speedup 1.0000x reference)
"""Trainium2 Bass kernel for nn_CLUB_816043786555 (CLUB loss).

Full-input contract: kernel(**inputs) takes the complete arrays, shards the
batch dim across 8 NeuronCores, runs a Bass/Tile kernel per core, and
combines tiny per-core partial sums on the host.

Math: with mu = leaky(x@W1m+b1m)@W2m+b2m, logvar = tanh(leaky(x@W1v+b1v)@W2v+b2v),
iv = exp(-logvar), ym_d = mean_i y, y2m_d = mean_i y^2:

  loss = -0.5/N * sum_{i,d} iv*((y^2 - y2m) - 2*mu*(y - ym))
       = -0.5/N * (P1 - 2*P2)

with yc = y - ym and y2c = y^2 - y2m centered ON THE HOST (exact, fp64) and
uploaded as fp16 alongside fp16 x — all pre-transposed to [feature, row]
blocked layout so the device does zero transposes and zero casts.  Per-core
partials P1 = sum iv*y2c and P2 = sum iv*mu*yc are accumulated per group via
engine accum_out in fp32; the host combine is O(NG) work.

Per 1024-row group the device runs a 4-engine balanced pipeline (~6.5-6.8us
each): PE does L1/L2 fp16 matmuls ([128,512] psum tiles, deep rotation);
leaky evacuations split 7/9 between ACT (Prelu w/ bias) and DVE (custom
fused bias+leaky uop); ACT also does tanh halves + exp; Pool does the mu
bias-add halves and the two accumulating product passes; DVE does the
q = iv*mu product as a 2x-mode fp16 tensor_tensor.
"""

import numpy as np

N_CORES = 8
N = 131072
D = 128
X_DIM = 128
H2 = 512
M = N // N_CORES          # rows per core = 16384
RG = 1024                 # rows per group
NG = M // RG              # groups per core = 16
NEG_SLOPE = 0.2

# Which of the 16 leaky evacuations per group run on ACT (True) vs DVE.
# 16 halves indexed (k, s, c) in PE emission order.
LEAKY_ON_ACT = tuple(i % 16 in (0, 2, 4, 7, 9, 11, 13) for i in range(16))

# Input prefetch depth (tile pool bufs for xt/yct/y2ct)
PREFETCH = 3
HPSUM_BUFS = 4
L2PSUM_BUFS = 2

_leaky_op = None


def _get_leaky_op():
    """Custom DVE uop: out = max((in0 + s0) * imm2, in0 + s0) — fused
    bias-add + leaky-relu in one 1x pass straight from PSUM."""
    global _leaky_op
    if _leaky_op is not None:
        return _leaky_op
    import concourse.dve_ops as DO
    from concourse.dve_spec import C0, C2, Spec, Src0, maxx

    op = DO.DveOp(
        "LEAKY_BIAS_ANT",
        Spec(
            body=maxx((Src0 + C0) * C2, Src0 + C0),
            reference=lambda in0, in1, s0, s1, imm2: np.maximum(
                (in0.astype(np.float32) + s0) * imm2,
                in0.astype(np.float32) + s0),
        ),
        subdim=False,
        uops_sha={"v3": "28ce115f5da0f06f", "v4": ""},
    )
    DO.OPS.append(op)
    DO.CUSTOM_DVE_SPECS[op.name] = op.spec
    DO._SUB_OPCODE_FOR_NAME[op.name] = DO._CUSTOM_DVE_ROW_BASE + len(DO.OPS) - 1
    assert DO._SUB_OPCODE_FOR_NAME[op.name] < 0x20
    _leaky_op = op
    return op


_compiled = None


def _build():
    import concourse.bacc as bacc
    import concourse.tile as tile
    import concourse.mybir as mybir

    F32 = mybir.dt.float32
    F16 = mybir.dt.float16
    AF = mybir.ActivationFunctionType
    OP = mybir.AluOpType

    nc = bacc.Bacc("TRN2", target_bir_lowering=False, debug=False,
                   num_devices=N_CORES)

    xt_d = nc.dram_tensor("xt", [NG, X_DIM, RG], F16, kind="ExternalInput")
    yct_d = nc.dram_tensor("yct", [NG, D, RG], F16, kind="ExternalInput")
    y2ct_d = nc.dram_tensor("y2ct", [NG, D, RG], F16, kind="ExternalInput")
    w1_d = [nc.dram_tensor("w1m", [X_DIM, H2], F16, kind="ExternalInput"),
            nc.dram_tensor("w1v", [X_DIM, H2], F16, kind="ExternalInput")]
    # pre-chunked on host: [4, 128, 128] with [c, h, d] = W2[c*128+h, d]
    w2_d = [nc.dram_tensor("w2m", [4, 128, D], F16, kind="ExternalInput"),
            nc.dram_tensor("w2v", [4, 128, D], F16, kind="ExternalInput")]
    # [128, 4] f32, column c = b1[c*128:(c+1)*128]
    b1_d = [nc.dram_tensor("b1m", [128, 4], F32, kind="ExternalInput"),
            nc.dram_tensor("b1v", [128, 4], F32, kind="ExternalInput")]
    # [128, 2] f32: col0 = b2m, col1 = -b2v
    b2_d = nc.dram_tensor("b2", [128, 2], F32, kind="ExternalInput")
    # columns 0..NG = per-group P1 (last group split in two), NG+1..2*NG+1 = P2
    out_d = nc.dram_tensor("out", [D, 2 * NG + 2], F32, kind="ExternalOutput")

    with tile.TileContext(nc) as tc:
        with (
            tc.tile_pool(name="singles", bufs=1) as singles,
            tc.tile_pool(name="inp", bufs=PREFETCH) as inp,
            tc.tile_pool(name="hidden", bufs=2) as hidden,
            tc.tile_pool(name="l2", bufs=2) as l2pool,
            tc.tile_pool(name="scratch", bufs=2) as scratch,
            tc.tile_pool(name="hpsum", bufs=HPSUM_BUFS, space="PSUM") as hpsum,
            tc.tile_pool(name="l2psum", bufs=L2PSUM_BUFS, space="PSUM") as l2psum,
        ):
            # ---- group-0 x first so the first matmul starts ASAP ----
            def load_x(g, split=False):
                xt = inp.tile([X_DIM, RG], F16, tag="xt", name="xt")
                if split:  # halves land sooner; first L1 reads [:, 0:512]
                    nc.sync.dma_start(xt[:, 0:512], xt_d[g, :, 0:512])
                    nc.sync.dma_start(xt[:, 512:RG], xt_d[g, :, 512:RG])
                else:
                    nc.sync.dma_start(xt[:], xt_d[g])
                return xt

            def load_y(g):
                yct = inp.tile([D, RG], F16, tag="yct", name="yct")
                y2ct = inp.tile([D, RG], F16, tag="y2ct", name="y2ct")
                nc.sync.dma_start(yct[:], yct_d[g])
                nc.sync.dma_start(y2ct[:], y2ct_d[g])
                return yct, y2ct

            xtiles = {0: load_x(0, split=True)}

            # ---- weights / biases via Pool SWDGE (keeps HWDGE free) ----
            w1t, w2t, b1t = [], [], []
            for k in range(2):
                w1 = singles.tile([X_DIM, H2], F16, tag=f"w1_{k}")
                nc.gpsimd.dma_start(w1[:], w1_d[k][:])
                w1t.append(w1)
                b1 = singles.tile([128, 4], F32, tag=f"b1_{k}")
                nc.gpsimd.dma_start(b1[:], b1_d[k][:])
                b1t.append(b1)
                w2 = singles.tile([128, 4, D], F16, tag=f"w2_{k}")
                nc.gpsimd.dma_start(w2[:], w2_d[k][:].rearrange("c h d -> h c d"))
                w2t.append(w2)
            b2 = singles.tile([128, 2], F32, tag="b2")
            nc.gpsimd.dma_start(b2[:], b2_d[:])

            ytiles = {0: load_y(0)}
            for g in range(1, min(PREFETCH - 1, NG)):
                xtiles[g] = load_x(g)
                ytiles[g] = load_y(g)

            acc = singles.tile([D, 2 * NG + 2], F32, tag="acc")

            for g in range(NG):
                xt = xtiles.pop(g)
                yct, y2ct = ytiles.pop(g)
                if g + PREFETCH - 1 < NG:
                    gp = g + PREFETCH - 1
                    xtiles[gp] = load_x(gp)
                    ytiles[gp] = load_y(gp)

                # hT[k][c] : [128, RG] f16 leaky(L1) output
                hT = [[hidden.tile([128, RG], F16, tag=f"hT{k}{c}",
                                   name=f"hT{k}{c}")
                       for c in range(4)] for k in range(2)]

                nleaky = 0

                def l1(k):
                    nonlocal nleaky
                    # halves ordered s-major so L2's s=0 deps complete first
                    for s in range(RG // 512):
                        for c in range(4):
                            hp = hpsum.tile([128, 512], F32, tag="hps")
                            nc.tensor.matmul(hp[:],
                                             w1t[k][:, c * 128:(c + 1) * 128],
                                             xt[:, s * 512:(s + 1) * 512],
                                             start=True, stop=True)
                            dst = hT[k][c][:, s * 512:(s + 1) * 512]
                            if LEAKY_ON_ACT[nleaky]:
                                nc.scalar.activation(
                                    dst, hp[:], AF.Prelu,
                                    bias=b1t[k][:, c:c + 1], scale=1.0,
                                    alpha=NEG_SLOPE)
                            else:
                                nc.vector._custom_dve(
                                    _get_leaky_op(), out=dst, in0=hp[:],
                                    s0=b1t[k][:, c:c + 1], imm2=NEG_SLOPE)
                            nleaky += 1

                def l2(k, s, ps):
                    for c in range(4):
                        nc.tensor.matmul(ps[:],
                                         w2t[k][:, c, :],
                                         hT[k][c][:, s * 512:(s + 1) * 512],
                                         start=(c == 0), stop=(c == 3))

                # --- all L1 first: leakies get ~3.4us of slack before L2
                mu16 = l2pool.tile([D, RG], F16, tag="mu16")
                u16 = l2pool.tile([D, RG], F16, tag="u16")
                l1(0)
                l1(1)
                for s in range(RG // 512):
                    mups = l2psum.tile([D, 512], F32, tag="mups")
                    l2(0, s, mups)
                    # mu evac on Pool frees the psum half early
                    nc.gpsimd.tensor_scalar(
                        out=mu16[:, s * 512:(s + 1) * 512], in0=mups[:],
                        scalar1=b2[:, 0:1], scalar2=None, op0=OP.add)
                for s in range(RG // 512):
                    zps = l2psum.tile([D, 512], F32, tag="zps")
                    l2(1, s, zps)
                    # u = tanh(-z - b2v) = -tanh(z + b2v)
                    nc.scalar.activation(u16[:, s * 512:(s + 1) * 512],
                                         zps[:], AF.Tanh,
                                         bias=b2[:, 1:2], scale=-1.0)

                iv16 = l2pool.tile([D, RG], F16, tag="iv16")
                q16 = scratch.tile([D, RG], F16, tag="q16")
                p1o = scratch.tile([D, RG], F16, tag="p1o")
                p2o = scratch.tile([D, RG], F16, tag="p2o")

                # last group: half-grain passes pipeline the drain chain
                tail = (q16, p1o, p2o)
                for lo, hi in ([(0, 512), (512, RG)] if g == NG - 1
                               else [(0, RG)]):
                    sl = slice(lo, hi)
                    nc.scalar.activation(iv16[:, sl], u16[:, sl], AF.Exp)
                    # q on DVE (fp16 2x mode), p1/p2 on Pool w/ accum
                    nc.vector.tensor_tensor(out=q16[:, sl], in0=iv16[:, sl],
                                            in1=mu16[:, sl], op=OP.mult)
                    col = g if lo == 0 else NG
                    nc.gpsimd.scalar_tensor_tensor(
                        out=p1o[:, sl], in0=iv16[:, sl], scalar=1.0,
                        in1=y2ct[:, sl], op0=OP.mult, op1=OP.mult,
                        accum_out=acc[:, col:col + 1])
                    col = NG + 1 + g if lo == 0 else 2 * NG + 1
                    nc.gpsimd.scalar_tensor_tensor(
                        out=p2o[:, sl], in0=q16[:, sl], scalar=1.0,
                        in1=yct[:, sl], op0=OP.mult, op1=OP.mult,
                        accum_out=acc[:, col:col + 1])

            nc.sync.dma_start(out_d[:], acc[:])

    nc.compile()
    return nc


def _get_compiled():
    global _compiled
    if _compiled is None:
        _compiled = _build()
    return _compiled


def _prep_host(x_samples, y_samples, W1m, b1m, W2m, b2m, W1v, b1v, W2v, b2v):
    """Host-side preprocessing: center y exactly (fp64), cast to fp16,
    transpose to [feature, row] and block into [NG, 128, RG] per core."""
    x = np.ascontiguousarray(x_samples, dtype=np.float32)
    y = np.asarray(y_samples, dtype=np.float32)
    y64 = y.astype(np.float64)
    ym = y64.mean(axis=0)
    y2m = (y64 * y64).mean(axis=0)
    yc = (y64 - ym).astype(np.float32)
    y2c = (y64 * y64 - y2m).astype(np.float32)

    def block(a):  # [M, 128] f32 -> [NG, 128, RG] f16 (transposed per group)
        return np.ascontiguousarray(
            a.reshape(NG, RG, 128).transpose(0, 2, 1)).astype(np.float16)

    w1m16 = np.asarray(W1m, np.float32).astype(np.float16)
    w1v16 = np.asarray(W1v, np.float32).astype(np.float16)

    def chunk_w2(W2):  # [512, 128] -> [4, 128, 128] f16
        return np.ascontiguousarray(
            np.asarray(W2, np.float32).reshape(4, 128, D)).astype(np.float16)

    def chunk_b1(b1):  # [512] -> [128, 4] f32, col c = b1[c*128:(c+1)*128]
        return np.ascontiguousarray(
            np.asarray(b1, np.float32).reshape(4, 128).T)

    b2c = np.ascontiguousarray(np.stack(
        [np.asarray(b2m, np.float32), -np.asarray(b2v, np.float32)],
        axis=1))

    shared = {
        "w1m": w1m16, "w1v": w1v16,
        "w2m": chunk_w2(W2m), "w2v": chunk_w2(W2v),
        "b1m": chunk_b1(b1m), "b1v": chunk_b1(b1v),
        "b2": b2c,
    }
    in_maps = []
    for i in range(N_CORES):
        sl = slice(i * M, (i + 1) * M)
        m = {"xt": block(x[sl]), "yct": block(yc[sl]),
             "y2ct": block(y2c[sl])}
        m.update(shared)
        in_maps.append(m)
    return in_maps


def kernel(x_samples, y_samples, W1m, b1m, W2m, b2m, W1v, b1v, W2v, b2v):
    from concourse.bass_utils import run_bass_kernel_spmd

    nc = _get_compiled()
    in_maps = _prep_host(x_samples, y_samples, W1m, b1m, W2m, b2m,
                         W1v, b1v, W2v, b2v)
    res = run_bass_kernel_spmd(nc, in_maps, list(range(N_CORES)))
    return combine([r["out"] for r in res.results])


def combine(outs):
    """Host-side gather: sum per-core [128, 2*NG+2] partials, finish the loss."""
    tot = np.sum([o.astype(np.float64) for o in outs], axis=(0, 1))
    P1 = tot[:NG + 1].sum()
    P2 = tot[NG + 1:].sum()
    return np.float32(-0.5 * (P1 - 2.0 * P2) / N)
